# revision 5
# baseline (speedup 1.0000x reference)
"""Trainium2 Bass kernel for soft decision-tree histogram binning.

Computes out[b, j] = prod_f softmax(x[b,f]*W + b_f, T=0.1)[digit_f(j)]
for x (4096, 7), cutpoints (7, 3) -> out (4096, 4**7=16384) float32.

Strategy (data-parallel over batch, 8 cores x 512 rows):
  - per-feature bias b_f from a 3-element min/mid/max sort of cutpoints,
    computed redundantly on all 128 partitions (no cross-partition traffic)
  - stabilized unnormalized e = exp((h - max_d h)/T) on the tiny (128, 28)
    tile; all 7 softmax denominators folded into one per-row scale
    C = 1/prod_f Z_f applied in the last cascade stage
  - output built as a Kronecker cascade of per-partition broadcast
    multiplies (tensor_scalar_mul, 2x DVE mode) 4 -> 16 -> ... -> 4096,
    final 4096->16384 stage split across ScalarE and VectorE
  - 2 MiB output DMAs per (row-tile, d0) block; DMA-bound at ~32 MiB/core
"""

import numpy as np

B = 4096
F = 7
D1 = 4  # D+1 bins per feature
OUT = D1**F  # 16384
NCORES = 8
ROWS = B // NCORES  # 512
P = 128
NTILES = ROWS // P  # 4
INV_T = 10.0

_cache = {}


def _build_bass():
    import concourse.bacc as bacc
    import concourse.tile as tile
    from concourse import mybir

    f32 = mybir.dt.float32
    Alu = mybir.AluOpType
    Act = mybir.ActivationFunctionType
    AX = mybir.AxisListType.X

    nc = bacc.Bacc("TRN2", target_bir_lowering=False, debug=False)

    x_d = nc.dram_tensor("x", [ROWS, F], f32, kind="ExternalInput").ap()
    cp_d = nc.dram_tensor("cutpoints", [F, 3], f32, kind="ExternalInput").ap()
    w_d = nc.dram_tensor("wpat", [F * D1], f32, kind="ExternalInput").ap()
    out_d = nc.dram_tensor("out", [ROWS, OUT], f32, kind="ExternalOutput").ap()

    with tile.TileContext(nc) as tc:
        with (
            tc.tile_pool(name="const", bufs=1) as cpool,
            tc.tile_pool(name="small", bufs=3) as sp,
            tc.tile_pool(name="mid", bufs=2) as mp,
            tc.tile_pool(name="big", bufs=2) as bigp,
            tc.tile_pool(name="blk", bufs=6) as blkp,
        ):
            # ---- per-core prep: W pattern + per-feature biases, all partitions ----
            wrep = cpool.tile([P, F * D1], f32)
            nc.sync.dma_start(out=wrep, in_=w_d[None, :].to_broadcast((P, F * D1)))
            w4 = wrep.rearrange("p (f d) -> p f d", d=D1)

            cpb = cpool.tile([P, F * 3], f32)
            nc.sync.dma_start(
                out=cpb,
                in_=cp_d.rearrange("a b -> (a b)")[None, :].to_broadcast((P, F * 3)),
            )
            cp3 = cpb.rearrange("p (f c) -> p f c", c=3)

            vmin = cpool.tile([P, F], f32)
            vmax = cpool.tile([P, F], f32)
            vsum = cpool.tile([P, F], f32)
            nc.vector.tensor_reduce(out=vmax, in_=cp3, axis=AX, op=Alu.max)
            nc.vector.tensor_reduce(out=vmin, in_=cp3, axis=AX, op=Alu.min)
            nc.vector.tensor_reduce(out=vsum, in_=cp3, axis=AX, op=Alu.add)

            # b_f = [0, -min, max-sum, -sum] per feature (cumsum of -sorted cuts)
            brep = cpool.tile([P, F * D1], f32)
            b4 = brep.rearrange("p (f d) -> p f d", d=D1)
            nc.vector.memset(b4[:, :, 0], 0.0)
            nc.vector.tensor_scalar_mul(out=b4[:, :, 1], in0=vmin, scalar1=-1.0)
            nc.vector.tensor_tensor(out=b4[:, :, 2], in0=vmax, in1=vsum, op=Alu.subtract)
            nc.vector.tensor_scalar_mul(out=b4[:, :, 3], in0=vsum, scalar1=-1.0)

            for t in range(NTILES):
                rows = slice(t * P, (t + 1) * P)

                xt = sp.tile([P, F], f32, tag="xt")
                nc.sync.dma_start(out=xt, in_=x_d[rows, :])

                # h[p, f, d] = x[p,f]*W[d] + b[f,d]
                h = sp.tile([P, F * D1], f32, tag="h")
                h4 = h.rearrange("p (f d) -> p f d", d=D1)
                xb = xt[:, :, None].broadcast_to((P, F, D1))
                nc.vector.tensor_tensor(out=h4, in0=xb, in1=w4, op=Alu.mult)
                nc.vector.tensor_tensor(out=h4, in0=h4, in1=b4, op=Alu.add)

                # stabilize: h -= max_d h
                m7 = sp.tile([P, F], f32, tag="m7")
                nc.vector.tensor_reduce(out=m7, in_=h4, axis=AX, op=Alu.max)
                mb = m7[:, :, None].broadcast_to((P, F, D1))
                nc.vector.tensor_tensor(out=h4, in0=h4, in1=mb, op=Alu.subtract)

                # e = exp(h / T), entries in (0, 1]
                e = sp.tile([P, F * D1], f32, tag="e")
                nc.scalar.activation(out=e, in_=h, func=Act.Exp, scale=INV_T)
                e4 = e.rearrange("p (f d) -> p f d", d=D1)

                # C = 1 / prod_f Z_f  (Z_f = sum_d e)
                z7 = sp.tile([P, F], f32, tag="z7")
                nc.vector.tensor_reduce(out=z7, in_=e4, axis=AX, op=Alu.add)
                q4 = sp.tile([P, 4], f32, tag="q4")
                nc.vector.tensor_tensor(
                    out=q4[:, 0:3], in0=z7[:, 0:3], in1=z7[:, 3:6], op=Alu.mult
                )
                nc.vector.tensor_copy(out=q4[:, 3:4], in_=z7[:, 6:7])
                q2 = sp.tile([P, 2], f32, tag="q2")
                nc.vector.tensor_tensor(
                    out=q2, in0=q4[:, 0:2], in1=q4[:, 2:4], op=Alu.mult
                )
                zp = sp.tile([P, 1], f32, tag="zp")
                nc.vector.tensor_tensor(
                    out=zp, in0=q2[:, 0:1], in1=q2[:, 1:2], op=Alu.mult
                )
                c1 = sp.tile([P, 1], f32, tag="c1")
                nc.vector.reciprocal(out=c1, in_=zp)

                # sc[:, d0] = e[:, f=0, d0] * C  (feature-0 factor + all denoms)
                sc = sp.tile([P, D1], f32, tag="sc")
                nc.vector.tensor_scalar_mul(out=sc, in0=e[:, 0:D1], scalar1=c1)

                # ---- Kronecker cascade: features 6,5 -> ... -> 1, then 0 ----
                t2 = sp.tile([P, 16], f32, tag="t2")
                for d in range(D1):
                    nc.vector.tensor_scalar_mul(
                        out=t2[:, d * 4 : (d + 1) * 4],
                        in0=e[:, 24:28],
                        scalar1=e[:, 20 + d : 21 + d],
                    )
                t3 = sp.tile([P, 64], f32, tag="t3")
                for d in range(D1):
                    nc.vector.tensor_scalar_mul(
                        out=t3[:, d * 16 : (d + 1) * 16],
                        in0=t2,
                        scalar1=e[:, 16 + d : 17 + d],
                    )
                t4 = sp.tile([P, 256], f32, tag="t4")
                for d in range(D1):
                    nc.vector.tensor_scalar_mul(
                        out=t4[:, d * 64 : (d + 1) * 64],
                        in0=t3,
                        scalar1=e[:, 12 + d : 13 + d],
                    )
                t5 = mp.tile([P, 1024], f32, tag="t5")
                for d in range(D1):
                    nc.vector.tensor_scalar_mul(
                        out=t5[:, d * 256 : (d + 1) * 256],
                        in0=t4,
                        scalar1=e[:, 8 + d : 9 + d],
                    )
                t6 = bigp.tile([P, 4096], f32, tag="t6")
                for d in range(D1):
                    nc.vector.tensor_scalar_mul(
                        out=t6[:, d * 1024 : (d + 1) * 1024],
                        in0=t5,
                        scalar1=e[:, 4 + d : 5 + d],
                    )

                # final stage: out block d0 = t6 * sc[:, d0]; 2 on ACT, 2 on DVE
                for d0 in range(D1):
                    blk = blkp.tile([P, 4096], f32, tag="blk")
                    if d0 < 2:
                        nc.scalar.mul(out=blk, in_=t6, mul=sc[:, d0 : d0 + 1])
                    else:
                        nc.vector.tensor_scalar_mul(
                            out=blk, in0=t6, scalar1=sc[:, d0 : d0 + 1]
                        )
                    nc.sync.dma_start(
                        out=out_d[rows, d0 * 4096 : (d0 + 1) * 4096], in_=blk
                    )
    nc.compile()
    return nc


def kernel(x, cutpoints):
    from concourse import bass_utils

    if "nc" not in _cache:
        _cache["nc"] = _build_bass()
    nc = _cache["nc"]

    x = np.ascontiguousarray(np.asarray(x), dtype=np.float32)
    cutpoints = np.ascontiguousarray(np.asarray(cutpoints), dtype=np.float32)
    wpat = np.tile(np.arange(1.0, D1 + 1.0, dtype=np.float32), F)

    in_maps = [
        {
            "x": np.ascontiguousarray(x[k * ROWS : (k + 1) * ROWS]),
            "cutpoints": cutpoints,
            "wpat": wpat,
        }
        for k in range(NCORES)
    ]
    res = bass_utils.run_bass_kernel_spmd(nc, in_maps, list(range(NCORES))).results
    return np.concatenate([res[k]["out"] for k in range(NCORES)], axis=0)
